# revision 11
# baseline (speedup 1.0000x reference)
"""AWGN channel kernel for Trainium2: y = x + sqrt(1/SNR) * noise.

Full inputs x, noise: (16384, 4096) float32. Row-sharded across 8
NeuronCores (data parallel, 2048 rows/core); each core streams 16
chunks of [128, 4096] through SBUF and computes the fused
(noise * STD) + x in one DVE scalar_tensor_tensor op per chunk.
"""

import numpy as np

N_CORES = 8
ROWS, COLS = 16384, 4096
SHARD_ROWS = ROWS // N_CORES  # 2048
P = 128
N_CHUNKS = SHARD_ROWS // P  # 16
SNR = 10.0
STD = float(np.sqrt(1.0 / SNR))

import os

CHUNK_COLS = int(os.environ.get("K_CHUNK_COLS", "4096"))
BUFS = int(os.environ.get("K_BUFS", "3"))
LOAD_ENGINES = os.environ.get("K_LOAD_ENGINES", "sync,sync")  # x,noise
STORE_ENGINE = os.environ.get("K_STORE_ENGINE", "scalar")

_cache = {}

IMPL = os.environ.get("K_IMPL", "tile")


def _build():
    if "nc" in _cache:
        return _cache["nc"]
    nc = _build_raw() if IMPL == "raw" else _build_tile()
    _cache["nc"] = nc
    return nc


def _build_raw():
    """Raw bass: manual per-engine programs + semaphores. Avoids Tile's
    entry/exit barrier machinery (~14us) at the cost of hand-rolled sync.

    SP engine issues all loads (x, noise), ACT issues all stores, DVE does
    the fused multiply-add. DMA completion is tracked Tile-style: a pool of
    semaphore lanes with at most one outstanding DMA per lane, round-robin.
    """
    import contextlib

    import concourse.bass as bass
    from concourse import mybir

    B = BUFS  # buffers per stream
    N_LANES = int(os.environ.get("K_LANES", "8"))

    nc = bass.Bass(
        "TRN2",
        target_bir_lowering=False,
        debug=False,
        num_devices=N_CORES,
    )
    x_ap = nc.dram_tensor(
        "x", [SHARD_ROWS, COLS], mybir.dt.float32, kind="ExternalInput"
    ).ap()
    n_ap = nc.dram_tensor(
        "noise", [SHARD_ROWS, COLS], mybir.dt.float32, kind="ExternalInput"
    ).ap()
    y_ap = nc.dram_tensor(
        "y", [SHARD_ROWS, COLS], mybir.dt.float32, kind="ExternalOutput"
    ).ap()

    x_v = x_ap.rearrange("(c p) f -> c p f", p=P)
    n_v = n_ap.rearrange("(c p) f -> c p f", p=P)
    y_v = y_ap.rearrange("(c p) f -> c p f", p=P)
    n_chunks = SHARD_ROWS // P  # 16

    xt = [
        nc.alloc_sbuf_tensor(f"xt{b}", [P, COLS], mybir.dt.float32).ap()
        for b in range(B)
    ]
    nt = [
        nc.alloc_sbuf_tensor(f"nt{b}", [P, COLS], mybir.dt.float32).ap()
        for b in range(B)
    ]
    yt = [
        nc.alloc_sbuf_tensor(f"yt{b}", [P, COLS], mybir.dt.float32).ap()
        for b in range(B)
    ]

    with contextlib.ExitStack() as ctx:
        load_lanes = [
            ctx.enter_context(nc.semaphore(f"ld{i}")) for i in range(N_LANES)
        ]
        store_lanes = [
            ctx.enter_context(nc.semaphore(f"st{i}")) for i in range(N_LANES // 2)
        ]
        sem_cmp = ctx.enter_context(nc.semaphore("cmp"))

        # python-side bookkeeping: lane -> number of DMAs issued so far
        ld_count = [0] * len(load_lanes)
        st_count = [0] * len(store_lanes)
        # per chunk: (lane_idx, threshold) for x / n loads and y stores
        x_done = [None] * n_chunks
        n_done = [None] * n_chunks
        s_done = [None] * n_chunks

        # pre-plan lane assignment round-robin in issue order
        for c in range(n_chunks):
            for rec, cnt, lanes in ((x_done, ld_count, load_lanes),
                                    (n_done, ld_count, load_lanes)):
                li = (2 * c + (0 if rec is x_done else 1)) % len(lanes)
                cnt[li] += 1
                rec[c] = (li, 16 * cnt[li])
        for c in range(n_chunks):
            li = c % len(store_lanes)
            st_count[li] += 1
            s_done[c] = (li, 16 * st_count[li])

        with nc.Block() as block:

            @block.sync
            def _(sync):
                for c in range(n_chunks):
                    b = c % B
                    if c >= B:
                        # x/n slot reuse: compute c-B must be done
                        sync.wait_ge(sem_cmp, c - B + 1)
                    for rec, src, dst in (
                        (x_done, x_v, xt),
                        (n_done, n_v, nt),
                    ):
                        li, thr = rec[c]
                        if thr > 16:
                            # lane reuse: previous DMA on lane done
                            sync.wait_ge(load_lanes[li], thr - 16)
                        sync.dma_start(out=dst[b][:], in_=src[c]).then_inc(
                            load_lanes[li], 16
                        )

            @block.vector
            def _(vector):
                for c in range(n_chunks):
                    b = c % B
                    xli, xthr = x_done[c]
                    nli, nthr = n_done[c]
                    vector.wait_ge(load_lanes[xli], xthr)
                    vector.wait_ge(load_lanes[nli], nthr)
                    if c >= B:
                        # y slot reuse: store c-B must be done
                        sli, sthr = s_done[c - B]
                        vector.wait_ge(store_lanes[sli], sthr)
                    vector.scalar_tensor_tensor(
                        out=yt[b][:],
                        in0=nt[b][:],
                        scalar=STD,
                        in1=xt[b][:],
                        op0=mybir.AluOpType.mult,
                        op1=mybir.AluOpType.add,
                    ).then_inc(sem_cmp, 1)

            @block.scalar
            def _(scalar):
                for c in range(n_chunks):
                    b = c % B
                    sli, sthr = s_done[c]
                    if sthr > 16:
                        scalar.wait_ge(store_lanes[sli], sthr - 16)
                    scalar.wait_ge(sem_cmp, c + 1)
                    scalar.dma_start(out=y_v[c], in_=yt[b][:]).then_inc(
                        store_lanes[sli], 16
                    )
                # do not finish until every store has landed in HBM
                for li, cnt in enumerate(st_count):
                    if cnt:
                        scalar.wait_ge(store_lanes[li], 16 * cnt)

    return nc


def _build_tile():
    import concourse.tile as tile
    from concourse import bacc, mybir

    nc = bacc.Bacc(
        "TRN2",
        target_bir_lowering=False,
        debug=False,
        num_devices=N_CORES,
    )
    x_ap = nc.dram_tensor(
        "x", [SHARD_ROWS, COLS], mybir.dt.float32, kind="ExternalInput"
    ).ap()
    n_ap = nc.dram_tensor(
        "noise", [SHARD_ROWS, COLS], mybir.dt.float32, kind="ExternalInput"
    ).ap()
    y_ap = nc.dram_tensor(
        "y", [SHARD_ROWS, COLS], mybir.dt.float32, kind="ExternalOutput"
    ).ap()

    eng_x, eng_n = (getattr(nc, e) for e in LOAD_ENGINES.split(","))
    eng_y = getattr(nc, STORE_ENGINE)

    # row-block view: block c = rows [128c, 128c+128) — fully contiguous 2 MiB
    x_v = x_ap.rearrange("(c p) f -> c p f", p=P)
    n_v = n_ap.rearrange("(c p) f -> c p f", p=P)
    y_v = y_ap.rearrange("(c p) f -> c p f", p=P)
    n_blocks = SHARD_ROWS // P  # 16

    # chunk schedule: (block, col_off, width). First/last blocks are split
    # into smaller column chunks to shrink pipeline fill/drain.
    def _splits(env, default):
        v = os.environ.get(env, default)
        out = [int(s) for s in v.split(",") if s]
        assert not out or sum(out) == COLS, out
        return out

    head = _splits("K_HEAD", "1024,1024,2048")
    tail = _splits("K_TAIL", "2048,1024,1024")
    chunks = []
    for c in range(n_blocks):
        if c == 0 and head:
            splits = head
        elif c == n_blocks - 1 and tail:
            splits = tail
        else:
            splits = [CHUNK_COLS] * (COLS // CHUNK_COLS)
        off = 0
        for w in splits:
            chunks.append((c, off, w))
            off += w

    # For the first EARLY_SPLIT chunks, split each load across both HWDGE
    # rings (sync + scalar) — descriptor generation is the ramp limiter at
    # kernel start, and the scalar ring is idle until the first store.
    early_split = int(os.environ.get("K_EARLY_SPLIT", "0"))

    with tile.TileContext(nc) as tc:
        with (
            tc.tile_pool(name="xp", bufs=BUFS) as xp,
            tc.tile_pool(name="npool", bufs=BUFS) as npool,
            tc.tile_pool(name="yp", bufs=BUFS) as yp,
        ):
            for ci, (c, off, w) in enumerate(chunks):
                xt = xp.tile([P, w], mybir.dt.float32, tag="xt")
                nt = npool.tile([P, w], mybir.dt.float32, tag="nt")
                yt = yp.tile([P, w], mybir.dt.float32, tag="yt")
                if ci < early_split:
                    h = w // 2
                    eng_x.dma_start(out=xt[:, :h], in_=x_v[c, :, off : off + h])
                    eng_y.dma_start(
                        out=xt[:, h:], in_=x_v[c, :, off + h : off + w]
                    )
                    eng_x.dma_start(out=nt[:, :h], in_=n_v[c, :, off : off + h])
                    eng_y.dma_start(
                        out=nt[:, h:], in_=n_v[c, :, off + h : off + w]
                    )
                else:
                    eng_x.dma_start(out=xt[:], in_=x_v[c, :, off : off + w])
                    eng_n.dma_start(out=nt[:], in_=n_v[c, :, off : off + w])
                nc.vector.scalar_tensor_tensor(
                    out=yt[:],
                    in0=nt[:],
                    scalar=STD,
                    in1=xt[:],
                    op0=mybir.AluOpType.mult,
                    op1=mybir.AluOpType.add,
                )
                eng_y.dma_start(out=y_v[c, :, off : off + w], in_=yt[:])

    nc.compile()
    _cache["nc"] = nc
    return nc


def _run(x, noise, trace=False, tmpdir=None):
    from concourse.bass_utils import run_bass_kernel_spmd

    nc = _build()
    x = np.ascontiguousarray(x, dtype=np.float32)
    noise = np.ascontiguousarray(noise, dtype=np.float32)
    in_maps = [
        {
            "x": x[i * SHARD_ROWS : (i + 1) * SHARD_ROWS],
            "noise": noise[i * SHARD_ROWS : (i + 1) * SHARD_ROWS],
        }
        for i in range(N_CORES)
    ]
    res = run_bass_kernel_spmd(
        nc, in_maps, list(range(N_CORES)), trace=trace, tmpdir=tmpdir
    )
    out = np.concatenate([res.results[i]["y"] for i in range(N_CORES)], axis=0)
    return out, res


def kernel(x, noise):
    out, _ = _run(x, noise)
    return out


# revision 17
# speedup vs baseline: 1.2015x; 1.2015x over previous
"""AWGN channel kernel for Trainium2: y = x + sqrt(1/SNR) * noise.

Full inputs x, noise: (16384, 4096) float32. Row-sharded across 8
NeuronCores (pure data parallel, 2048 rows/core, no communication).

Per core, the shard is streamed as 16 row-blocks of [128 rows x 4096
cols] (each a fully contiguous 2 MiB region — contiguity matters: a
strided DRAM layout measured ~18% slower). Loads go on the SP HWDGE
ring, stores on the ACT HWDGE ring, and the arithmetic is a single
fused DVE op per chunk: scalar_tensor_tensor computes
(noise * STD) + x in one pass, which is bit-exact vs the f32
reference (fp32 multiply-round then add-round, same as jax).

The kernel is DMA-bound: 96 MiB of HBM<->SBUF traffic per core at the
~435 GB/s SBUF-fabric ceiling = ~236 us; measured best-case exec is
~249 us (ramp + NEFF wrapper overhead accounts for the rest; the
steady state profiles at >99% DMA-engine occupancy with zero mid-span
gaps). The last row-block is split into two 2048-column chunks to
shorten the pipeline-drain tail.

Triple buffering (bufs=3) per stream: 3 pools x 3 bufs x 16 KiB =
144 KiB per SBUF partition.
"""

import numpy as np

N_CORES = 8
ROWS, COLS = 16384, 4096
SHARD_ROWS = ROWS // N_CORES  # 2048 rows per core
P = 128  # SBUF partitions
SNR = 10.0
STD = float(np.sqrt(1.0 / SNR))

BUFS = 3
TAIL_SPLITS = [2048, 2048]  # column splits of the last row-block

_cache = {}


def _build():
    if "nc" in _cache:
        return _cache["nc"]

    import concourse.tile as tile
    from concourse import bacc, mybir

    nc = bacc.Bacc(
        "TRN2",
        target_bir_lowering=False,
        debug=False,
        num_devices=N_CORES,
    )
    x_ap = nc.dram_tensor(
        "x", [SHARD_ROWS, COLS], mybir.dt.float32, kind="ExternalInput"
    ).ap()
    n_ap = nc.dram_tensor(
        "noise", [SHARD_ROWS, COLS], mybir.dt.float32, kind="ExternalInput"
    ).ap()
    y_ap = nc.dram_tensor(
        "y", [SHARD_ROWS, COLS], mybir.dt.float32, kind="ExternalOutput"
    ).ap()

    # row-block view: block c = rows [128c, 128c+128), fully contiguous 2 MiB
    x_v = x_ap.rearrange("(c p) f -> c p f", p=P)
    n_v = n_ap.rearrange("(c p) f -> c p f", p=P)
    y_v = y_ap.rearrange("(c p) f -> c p f", p=P)
    n_blocks = SHARD_ROWS // P  # 16

    chunks = [(c, 0, COLS) for c in range(n_blocks - 1)]
    off = 0
    for w in TAIL_SPLITS:
        chunks.append((n_blocks - 1, off, w))
        off += w
    assert off == COLS

    with tile.TileContext(nc) as tc:
        with (
            tc.tile_pool(name="xp", bufs=BUFS) as xp,
            tc.tile_pool(name="npool", bufs=BUFS) as npool,
            tc.tile_pool(name="yp", bufs=BUFS) as yp,
        ):
            for c, off, w in chunks:
                xt = xp.tile([P, w], mybir.dt.float32, tag="xt")
                nt = npool.tile([P, w], mybir.dt.float32, tag="nt")
                yt = yp.tile([P, w], mybir.dt.float32, tag="yt")
                nc.sync.dma_start(out=xt[:], in_=x_v[c, :, off : off + w])
                nc.sync.dma_start(out=nt[:], in_=n_v[c, :, off : off + w])
                nc.vector.scalar_tensor_tensor(
                    out=yt[:],
                    in0=nt[:],
                    scalar=STD,
                    in1=xt[:],
                    op0=mybir.AluOpType.mult,
                    op1=mybir.AluOpType.add,
                )
                nc.scalar.dma_start(out=y_v[c, :, off : off + w], in_=yt[:])

    nc.compile()
    _cache["nc"] = nc
    return nc


def _run(x, noise, trace=False, tmpdir=None):
    from concourse.bass_utils import run_bass_kernel_spmd

    nc = _build()
    x = np.ascontiguousarray(x, dtype=np.float32)
    noise = np.ascontiguousarray(noise, dtype=np.float32)
    in_maps = [
        {
            "x": x[i * SHARD_ROWS : (i + 1) * SHARD_ROWS],
            "noise": noise[i * SHARD_ROWS : (i + 1) * SHARD_ROWS],
        }
        for i in range(N_CORES)
    ]
    res = run_bass_kernel_spmd(
        nc, in_maps, list(range(N_CORES)), trace=trace, tmpdir=tmpdir
    )
    out = np.concatenate([res.results[i]["y"] for i in range(N_CORES)], axis=0)
    return out, res


def kernel(x, noise):
    out, _ = _run(x, noise)
    return out
